# revision 54
# baseline (speedup 1.0000x reference)
"""Tensor-parallel multi-head attention for Trainium2 (8 NeuronCores).

Reference computation (fp32):
    qkv = hidden @ w_qkv.T + b_qkv            # [B,S,3H]
    q,k,v = split/heads                       # [B,NH,S,HD]
    out_h = softmax(q k^T / sqrt(HD)) v       # [B,NH,S,HD]
    out = concat_heads(out_h) @ w_out.T + b_out

Sharding (Megatron-style tensor parallel over NH=16 heads, 2 heads/core):
  - hidden (transposed, [H, B*S]) replicated to all 8 cores
  - each core: QKV projection for its 2 heads -> attention for its 2 heads
    -> normalized context^T [256, 512] per 512-token chunk
  - per-chunk AllGather of context^T -> [2048, 512]
  - each core computes a disjoint 256-row slice of the output projection
  - host concatenates row slices: zero host FLOPs

v2 layout/schedule (vs the fp32r v1 baseline at ~715us; now ~535us):
  - bf16 everywhere on the PE (1 cyc/row, same rate as fp32r) which
    halves the AllGather payload, HBM traffic and SBUF footprint.
    Measured rel-l2 of the full bf16 pipeline: 5.5e-3 (gate: 2e-2).
  - batch-pipelined: QKV(b0) -> attn b0 (shipping each 512-token chunk's
    gather as it completes) -> QKV(b1) (projections of early chunks +
    batch-1 x prefetched underneath) -> attn b1 -> tail projections of
    the chunks held back to hide the last gathers.  Collectives overlap
    compute instead of piling up at the end (a per-chunk AllGather costs
    ~20us, mostly fixed cost, so exactly one gather per chunk).
  - qk^T/V buffers sized for ONE batch (Tile's write-after-read sems
    order QKV(b1) behind attn(b0)'s reads), halving their SBUF.
  - softmax denominators: two in-place pairwise bf16 tree-adds on the
    DVE over the stored exp tiles, then four accumulating ones-matmuls
    (the v1 per-k-tile ones-matmul chain cost ~33% of attention PE
    time); plain [1,512] reciprocal on DVE; inverse broadcast across
    partitions with a K=1 ones-row matmul.
  - attention runs a 3-stage software pipeline (attn -> denom -> norm,
    one head-step apart) so the reciprocal's ~3.3us latency and the
    broadcast never stall the PE; the unnormalized context is copied
    PSUM->SBUF immediately to recycle the accumulation bank.
  - exp activations cover two score tiles each (PSUM [P,2,512]); the
    Scalar engine is the binding resource of the attention phases at
    ~9.2us per head-step (near its element-rate roofline).
  - Tile does NOT track DRAM written by collectives: every read of a
    gathered chunk carries an explicit dep edge on its AllGather
    (without it the projection races the gather and corrupts output).
"""

import sys

sys.path.insert(0, "/opt/trn_rl_repo")

import numpy as np
import ml_dtypes

import concourse.bass as bass
import concourse.tile as tile
from concourse import mybir
from concourse.bass_utils import run_bass_kernel_spmd
from concourse.tile import ScopedClock
from concourse.tile_rust import add_dep_helper

FP32 = mybir.dt.float32
F32R = mybir.dt.float32r
BF16 = mybir.dt.bfloat16

B = 2
S = 2048
H = 2048
NH = 16
HD = 128
N_CORES = 8
HPC = NH // N_CORES  # heads per core = 2
T = B * S  # 4096
O_QK = 2 * HPC * HD  # 512 rows of qk^T per core (Q then K)
O_V = HPC * HD  # 256
O_OUT = H // N_CORES  # 256 output rows per core
SCALE = 1.0 / float(np.sqrt(HD))
P = 128

MM_DT = BF16

MAX_WAITS = 1  # the pinned walrus codegen rejects >1 sync wait per inst


def _wait_limit(inst):
    return MAX_WAITS


class _TileContext(tile.TileContext):
    """Tile patched for the pinned walrus codegen's sync-wait limit.

    Any instruction carrying more than MAX_WAITS semaphore waits is split:
    preceding same-engine nops carry the excess (engines execute their
    stream in order, so the waits still all precede the instruction).
    """

    def _lower_ordered_insts(self, ordered):
        nc = self.nc
        for bb_name, insts in list(ordered.items()):
            new_insts = []
            for inst in insts:
                si = inst.sync_info
                limit = _wait_limit(inst)
                if (
                    si is not None
                    and len(si.on_wait) > limit
                    and inst.engine is not None
                ):
                    waits = list(si.on_wait)
                    while len(waits) > limit:
                        chunk, waits = waits[:limit], waits[limit:]
                        new_insts.append(
                            mybir.InstNoOp(
                                name=nc.get_next_instruction_name(),
                                sync_info=mybir.SyncInfo(
                                    on_wait=chunk, on_update=[]
                                ),
                                bass_nofuse=True,
                                engine=inst.engine,
                            )
                        )
                    inst.sync_info = mybir.SyncInfo(
                        on_wait=waits, on_update=list(si.on_update)
                    )
                new_insts.append(inst)
            ordered[bb_name] = new_insts
        return super()._lower_ordered_insts(ordered)

    def _drain_and_barrier(self, tick_clock, wait_clock):
        nc = self.nc
        probe = nc.sync.nop(nofuse=True, hint="drain_wait_probe")
        wait_clock.add_sem_waits(
            probe.ins, ScopedClock({None: tick_clock.global_clock})
        )
        si = probe.ins.sync_info
        waits = list(si.on_wait) if si is not None else []
        probe.ins.sync_info = mybir.SyncInfo(
            on_wait=[], on_update=list(si.on_update) if si else []
        )
        for w in waits:
            n = nc.sync.nop(nofuse=True, hint="drain_wait_split")
            n.ins.sync_info = mybir.SyncInfo(on_wait=[w], on_update=[])
        nc.sync.drain()
        nc.all_engine_barrier()
        assert self.sems is not None
        popped = nc._tile_sem_poison_stack.pop()
        assert popped is self._sem_poison
        nc.clear_and_free_semaphores(list(self.sems.allocated().values()))
        nc.all_engine_barrier()


def _build_program(seq=S, mm_dt=MM_DT):
    """Build the SPMD Bass program (identical on all 8 cores)."""
    t_all = B * seq
    n_ht = H // P  # 16 k-tiles over the hidden dim
    ts_w = 512  # token-slice width for the QKV stage
    n_ts = seq // ts_w  # slices per batch
    qs_w = 512  # q-slice width in attention
    n_qs = seq // qs_w
    n_kt = seq // P  # k tiles per batch in attention
    n_dt = H // P  # d tiles of the gathered context
    n_ch = B * n_qs  # token chunks, gathered + projected as they finish

    nc = bass.Bass(
        "TRN2", target_bir_lowering=False, debug=False, num_devices=N_CORES
    )

    # pre-tiled on host to [partition, k-tile, free] so each DMA descriptor
    # covers a partition's full contiguous row
    xt = nc.dram_tensor("xt", [P, n_ht, t_all], mm_dt, kind="ExternalInput")
    w1t_qk = nc.dram_tensor(
        "w1t_qk", [P, n_ht, O_QK], mm_dt, kind="ExternalInput"
    )
    w1t_v = nc.dram_tensor("w1t_v", [P, n_ht, O_V], mm_dt, kind="ExternalInput")
    b_qk = nc.dram_tensor("b_qk", [P, O_QK // P], FP32, kind="ExternalInput")
    b_v = nc.dram_tensor("b_v", [P, O_V], FP32, kind="ExternalInput")
    wout_t = nc.dram_tensor(
        "wout_t", [P, n_dt, O_OUT], mm_dt, kind="ExternalInput"
    )
    b_out = nc.dram_tensor("b_out", [P, O_OUT // P], FP32, kind="ExternalInput")
    # ones loaded, not memset: the pinned walrus rejects f32r/bf16 memset
    ones_c = nc.dram_tensor("ones_c", [P, 1], mm_dt, kind="ExternalInput")
    ones_r = nc.dram_tensor("ones_r", [1, P], mm_dt, kind="ExternalInput")
    out = nc.dram_tensor("out", [O_OUT, t_all], FP32, kind="ExternalOutput")

    # one gather per chunk: collectives carry a ~15-20us fixed cost, so
    # fewer/bigger gathers keep the CC queue from backing up
    cc_in = nc.dram_tensor("cc_in", [n_ch, O_V, qs_w], mm_dt)
    cc_out = nc.dram_tensor(
        "cc_out", [n_ch, H, qs_w], mm_dt, addr_space="Shared"
    )

    xt_r = xt.ap()
    w1t_qk_r = w1t_qk.ap()
    w1t_v_r = w1t_v.ap()
    wout_r = wout_t.ap()
    cc_in_r = cc_in.ap().rearrange("c (h p) t -> c p h t", p=P)
    cc_out_r = cc_out.ap().rearrange("c (dt p) t -> c p dt t", p=P)
    out_r = out.ap().rearrange("(ot p) t -> p ot t", p=P)

    MM = nc.tensor.matmul

    with _TileContext(nc) as tc:
        with (
            tc.tile_pool(name="const", bufs=1) as const,
            tc.tile_pool(name="wq", bufs=1) as wq,
            tc.tile_pool(name="ctxs", bufs=2) as ctxs,
            tc.tile_pool(name="outs", bufs=2) as outs,
            # one shared [P,qs_w] PSUM pool serves the attention context
            # accumulators, the inverse-broadcast matmuls and the output
            # projection (their lifetimes interleave; 3 bufs + the 4-bank
            # score pool + the 1-bank sum pool exactly fill the 8 banks)
            tc.tile_pool(name="ps_cb", bufs=3, space="PSUM") as ps_cb,
        ):
            # --- critical-path DMAs first: the first QKV matmul needs the
            # first weight chunk and the first x slice; everything else can
            # land later.
            WCH = 2  # k-tiles per weight DMA chunk
            w_qk_ch = []
            t0 = wq.tile([P, WCH, O_QK], mm_dt, name="w_qk_0")
            nc.sync.dma_start(t0[:], w1t_qk_r[:, 0:WCH, :])
            w_qk_ch.append(t0)

            acts_scope = tc.tile_pool(name="acts", bufs=1)
            acts = acts_scope.__enter__()

            xts_scope = tc.tile_pool(name="xts", bufs=2)
            xts = xts_scope.__enter__()
            xt_first = xts.tile([P, n_ht, ts_w], mm_dt)
            # interleave the first slice's strips with the weight chunks so
            # the QKV accumulation chain never outruns its weight loads
            for i in range(4):
                nc.sync.dma_start(
                    xt_first[:, 4 * i : 4 * i + 4, :],
                    xt_r[:, 4 * i : 4 * i + 4, 0:ts_w],
                )
                for j in (2 * i + 1, 2 * i + 2):
                    if j < n_ht // WCH:
                        t = wq.tile([P, WCH, O_QK], mm_dt, name=f"w_qk_{j}")
                        nc.sync.dma_start(
                            t[:], w1t_qk_r[:, j * WCH : (j + 1) * WCH, :]
                        )
                        w_qk_ch.append(t)
            w_v_ch = []
            for i in range(n_ht // WCH):
                t = wq.tile([P, WCH, O_V], mm_dt, name=f"w_v_{i}")
                nc.sync.dma_start(t[:], w1t_v_r[:, i * WCH : (i + 1) * WCH, :])
                w_v_ch.append(t)

            b_qk_sb = const.tile([P, O_QK // P], FP32)
            nc.sync.dma_start(b_qk_sb[:], b_qk.ap())
            b_v_sb = const.tile([P, O_V], FP32)
            nc.sync.dma_start(b_v_sb[:], b_v.ap())
            b_out_sb = const.tile([P, O_OUT // P], FP32)
            nc.sync.dma_start(b_out_sb[:], b_out.ap())
            ones_col = const.tile([P, 1], mm_dt)
            nc.sync.dma_start(ones_col[:], ones_c.ap())
            ones_row = const.tile([1, P], mm_dt)
            nc.sync.dma_start(ones_row[:], ones_r.ap())

            wout_sb = wq.tile([P, n_dt, O_OUT], mm_dt)
            nc.sync.dma_start(wout_sb[:], wout_r)

            # ---------------- persistent activations ----------------
            # single-batch sized: attn(b) consumes them before QKV(b+1)
            # overwrites (Tile's write-after-read sems order the phases)
            qk_sb = acts.tile([P, O_QK // P, seq], mm_dt)  # qk^T
            v_sb = acts.tile([P, seq // P, O_V], mm_dt)  # V natural

            # ---------------- stage helpers ----------------
            def qkv_slice(b, ts, xt_t, ps1, ps1v):
                """QKV projection for one 512-token slice of batch b."""
                lo = ts * ts_w  # batch-local activation offset
                for ot in range(O_QK // P):
                    ps = ps1.tile([P, ts_w], FP32)
                    for kt in range(n_ht):
                        MM(
                            ps[:],
                            w_qk_ch[kt // WCH][
                                :, kt % WCH, ot * P : (ot + 1) * P
                            ],
                            xt_t[:, kt, :],
                            start=(kt == 0),
                            stop=(kt == n_ht - 1),
                        )
                    nc.scalar.activation(
                        qk_sb[:, ot, lo : lo + ts_w],
                        ps[:],
                        mybir.ActivationFunctionType.Identity,
                        bias=b_qk_sb[:, ot : ot + 1],
                    )
                for tt in range(ts_w // P):
                    psv = ps1v.tile([P, O_V], FP32)
                    for kt in range(n_ht):
                        MM(
                            psv[:],
                            xt_t[:, kt, tt * P : (tt + 1) * P],
                            w_v_ch[kt // WCH][:, kt % WCH, :],
                            start=(kt == 0),
                            stop=(kt == n_ht - 1),
                        )
                    nc.vector.tensor_add(
                        v_sb[:, lo // P + tt, :], psv[:], b_v_sb[:]
                    )

            def proj_load(ch, eng=None):
                """Issue the gathered-context load for a chunk's
                projection well before the compute needs it (split in two
                DMAs so it spreads over queues).  Mid-run triggers ride the
                gpsimd queue (a gather-wait there only delays the next
                collective enqueue, which is gather-serialized anyway); the
                tail uses sync, where no ships remain to block."""
                hd = n_dt // 2
                t = ctxs.tile([P, n_dt, qs_w], mm_dt, name="ctx_t")
                for i in range(2):
                    dma = (eng or nc.gpsimd).dma_start(
                        t[:, i * hd : (i + 1) * hd, :],
                        cc_out_r[ch][:, i * hd : (i + 1) * hd, :],
                    )
                    # Tile does not track DRAM writes made by collectives;
                    # order the read behind the gather explicitly
                    add_dep_helper(
                        dma.ins, gathers[ch].ins,
                        reason="proj load waits for chunk gather",
                    )
                return t

            def proj_ot(ch, ctx_t, ot):
                """One 128-row slice of a chunk's output projection
                (a self-contained 16-matmul PSUM chain, usable as PE
                filler inside ACT/DVE-bound attention steps)."""
                b, qs = divmod(ch, n_qs)
                t_lo = b * seq + qs * qs_w
                ps_o = ps_cb.tile([P, qs_w], FP32, tag="cb", name="ps_o")
                for dt in range(n_dt):
                    MM(
                        ps_o[:],
                        wout_sb[:, dt, ot * P : (ot + 1) * P],
                        ctx_t[:, dt, :],
                        start=(dt == 0),
                        stop=(dt == n_dt - 1),
                    )
                out_t = outs.tile([P, qs_w], FP32, name="out_t")
                nc.scalar.activation(
                    out_t[:],
                    ps_o[:],
                    mybir.ActivationFunctionType.Identity,
                    bias=b_out_sb[:, ot : ot + 1],
                )
                nc.sync.dma_start(
                    out_r[:, ot, t_lo : t_lo + qs_w], out_t[:]
                )

            def proj_compute(ch, ctx_t):
                for ot in range(O_OUT // P):
                    proj_ot(ch, ctx_t, ot)

            gathers = {}

            def ship_chunk(ch, ctx_ch):
                nc.sync.dma_start(cc_in_r[ch], ctx_ch[:])
                gathers[ch] = nc.gpsimd.collective_compute(
                    "AllGather",
                    mybir.AluOpType.bypass,
                    replica_groups=[list(range(N_CORES))],
                    ins=[cc_in.ap()[ch]],
                    outs=[cc_out.ap()[ch]],
                )

            def attn_step(ch, h, pools, filler=None):
                """Attention for one head of one 512-q chunk.  Returns the
                state the (deferred) denominator + normalize stages need.
                `filler` emits PE work (a projection slice) into the gap
                where the PE would otherwise wait on the Scalar/Vector
                engines at the step boundary."""
                exps, sums, ctxu, ps_s, ps_r = pools
                b, qs = divmod(ch, n_qs)
                q_lo = qs * qs_w  # batch-local
                exps_t = exps.tile([P, n_kt, qs_w], mm_dt, name="exps_t")
                if filler is not None:
                    filler()
                ps_ctx = ps_cb.tile([P, qs_w], FP32, tag="cb", name="ps_ctx")
                for kt2 in range(n_kt // 2):
                    ps_sc = ps_s.tile([P, 2, qs_w], FP32, name="ps_sc")
                    for j in range(2):
                        kt = 2 * kt2 + j
                        k_lo = kt * P
                        MM(
                            ps_sc[:, j, :],
                            qk_sb[:, HPC + h, k_lo : k_lo + P],
                            qk_sb[:, h, q_lo : q_lo + qs_w],
                            start=True,
                            stop=True,
                        )
                    # one exp activation covers both score tiles
                    nc.scalar.activation(
                        exps_t[:, 2 * kt2 : 2 * kt2 + 2, :],
                        ps_sc[:],
                        mybir.ActivationFunctionType.Exp,
                        scale=SCALE,
                    )
                    for j in range(2):
                        kt = 2 * kt2 + j
                        MM(
                            ps_ctx[:],
                            v_sb[:, kt, h * HD : (h + 1) * HD],
                            exps_t[:, kt, :],
                            start=(kt == 0),
                            stop=(kt == n_kt - 1),
                        )
                # early PSUM->SBUF copy first in the DVE queue: it frees
                # the context bank for the pool as soon as possible
                ctx_u = ctxu.tile([P, qs_w], FP32, name="ctx_u")
                nc.vector.tensor_copy(ctx_u[:], ps_ctx[:])
                # denominator front half: in-place pairwise tree over the
                # kt axis (contiguous DVE reads; the ctx matmuls that read
                # each slice have already consumed it).  bf16 partials cost
                # ~0.3% on the denominators -- well inside the error budget.
                tree_last = None
                for w in (8, 4):
                    tree_last = nc.vector.tensor_add(
                        exps_t[:, 0:w, :],
                        exps_t[:, 0:w, :],
                        exps_t[:, w : 2 * w, :],
                    )
                return exps_t, ctx_u, tree_last

            def denom_step(ch, h, ctx_ch, exps_t, ctx_u, pools,
                           after=None):
                """One step behind attention: partition-reduce the exp sums
                and take the reciprocal (runs while the next head's
                attention keeps the PE busy)."""
                exps, sums, ctxu, ps_s, ps_r = pools
                # the tree's last two levels ride the partition-reduce:
                # four accumulating ones-matmuls (PE has slack; DVE doesn't)
                ps_sum = ps_r.tile([1, qs_w], FP32, name="ps_sum")
                for j in range(4):
                    MM(
                        ps_sum[:], ones_col[:], exps_t[:, j, :],
                        start=(j == 0), stop=(j == 3),
                    )
                inv = sums.tile([1, qs_w], mm_dt, name="inv")
                with nc.allow_low_precision(reason="bf16 inverse: ~0.2% on a"
                                            " 2e-2 budget"):
                    rec = nc.vector.reciprocal(inv[:], ps_sum[:])
                if after is not None:
                    # keep the 3.3us reciprocal OUT of the per-step DVE
                    # cycle: without this edge the scheduler slots it
                    # between the next step's tree adds, and the sum
                    # matmuls then stall the PE every step
                    add_dep_helper(
                        rec.ins, after.ins,
                        reason="recip after next step's tree",
                    )
                return ch, h, ctx_ch, ctx_u, inv

            def norm_step(ch, h, ctx_ch, ctx_u, inv):
                # two steps behind attention: by now the reciprocal has
                # finished, so the broadcast matmul never stalls the PE
                ps_b = ps_cb.tile([P, qs_w], FP32, tag="cb", name="ps_b")
                MM(ps_b[:], ones_row[:], inv[:], start=True, stop=True)
                nc.vector.tensor_mul(ctx_ch[:, h, :], ctx_u[:], ps_b[:])
                if h == HPC - 1:
                    ship_chunk(ch, ctx_ch)

            # ---------------- schedule ----------------
            ld = {}  # prefetched proj context tiles, by chunk

            def load_slice(xt_t, b, ts):
                # split across queues: a single-queue 2MB transfer takes
                # ~20us, which would stall the PE at phase boundaries
                lo = b * seq + ts * ts_w
                for i in range(4):
                    nc.sync.dma_start(
                        xt_t[:, 4 * i : 4 * i + 4, :],
                        xt_r[:, 4 * i : 4 * i + 4, lo : lo + ts_w],
                    )

            def qkv_batch(b, first_tile=None):
                with (
                    tc.tile_pool(name="ps1", bufs=2, space="PSUM") as ps1,
                    tc.tile_pool(name="ps1v", bufs=2, space="PSUM") as ps1v,
                ):
                    for ts in range(n_ts):
                        if ts == 0 and first_tile is not None:
                            xt_t = first_tile
                        else:
                            xt_t = xts.tile([P, n_ht, ts_w], mm_dt)
                            load_slice(xt_t, b, ts)
                        qkv_slice(b, ts, xt_t, ps1, ps1v)
                        # early chunks' projections ride inside the second
                        # QKV phase once their gathers have landed
                        if b == 1 and ts == 0:
                            ld[2] = proj_load(2)
                        if b == 1 and ts == 1:
                            proj_compute(2, ld.pop(2))
                            ld[3] = proj_load(3)
                        if b == 1 and ts == n_ts - 1:
                            proj_compute(3, ld.pop(3))

            def attn_batch(b):
                with (
                    tc.tile_pool(name="ctxp", bufs=3) as ctxp,
                    tc.tile_pool(name="exps", bufs=2) as exps,
                    tc.tile_pool(name="sums", bufs=2) as sums,
                    tc.tile_pool(name="ctxu", bufs=3) as ctxu,
                    tc.tile_pool(name="ps_s", bufs=2, space="PSUM") as ps_s,
                    tc.tile_pool(name="ps_r", bufs=1, space="PSUM") as ps_r,
                ):
                    pools = (exps, sums, ctxu, ps_s, ps_r)
                    pend_d = None  # awaiting denom_step
                    pend_n = None  # awaiting norm_step
                    for qs in range(n_qs):
                        ch = b * n_qs + qs
                        ctx_ch = ctxp.tile([P, HPC, qs_w], mm_dt, name="ctx_ch")
                        for h in range(HPC):
                            exps_t, ctx_u, tree = attn_step(ch, h, pools)
                            if pend_d is not None:
                                st = denom_step(*pend_d, pools, after=tree)
                                if pend_n is not None:
                                    norm_step(*pend_n)
                                pend_n = st
                            pend_d = (ch, h, ctx_ch, exps_t, ctx_u)
                        # prefetch the lagged projection's context one full
                        # chunk ahead of its compute (its ship was emitted
                        # within this chunk's step loop)
                        if b == 1 and 1 <= qs <= 2:
                            ld[qs + 3] = proj_load(qs + 3)
                        # lagged projection keeps the PE off the gather path
                        # (chunks 0 and 1 are held back as tail filler)
                        if b == 1 and qs >= 2:
                            pch = n_qs + qs - 2
                            proj_compute(pch, ld.pop(pch))
                    st = denom_step(*pend_d, pools)
                    if pend_n is not None:
                        norm_step(*pend_n)
                    norm_step(*st)

            qkv_batch(0, first_tile=xt_first)
            xts_scope.__exit__(None, None, None)
            # prefetch batch 1's first slice underneath attention b0 so the
            # second QKV phase starts without a DMA bubble
            xts_scope = tc.tile_pool(name="xts", bufs=2)
            xts = xts_scope.__enter__()
            xt_b1 = xts.tile([P, n_ht, ts_w], mm_dt)
            load_slice(xt_b1, 1, 0)
            attn_batch(0)
            qkv_batch(1, first_tile=xt_b1)
            xts_scope.__exit__(None, None, None)
            attn_batch(1)
            # tail: the held-back early chunks hide the last gathers
            ld0 = proj_load(0, eng=nc.sync)
            proj_compute(0, ld0)
            ld6 = proj_load(n_ch - 2, eng=nc.sync)
            ld1 = proj_load(1, eng=nc.sync)
            proj_compute(n_ch - 2, ld6)
            proj_compute(1, ld1)
            ld7 = proj_load(n_ch - 1, eng=nc.sync)
            proj_compute(n_ch - 1, ld7)

            acts_scope.__exit__(None, None, None)

    return nc


def _tile_rows(a, dt=None):
    """[H, F] -> [128, H//128, F] (row r = kt*128 + p becomes [p, kt])."""
    h, f = a.shape
    return np.ascontiguousarray(
        a.reshape(h // P, P, f).transpose(1, 0, 2),
        dtype=dt or ml_dtypes.bfloat16,
    )


def _make_in_maps(hidden_states, w_qkv, b_qkv, w_out, b_out):
    b, s, _ = hidden_states.shape
    t_all = b * s
    x = _tile_rows(
        np.ascontiguousarray(hidden_states.reshape(t_all, H).T, dtype=np.float32)
    )  # [P, H//P, T] bf16
    in_maps = []
    for c in range(N_CORES):
        h0 = HPC * c
        q_rows = np.r_[h0 * HD : (h0 + HPC) * HD]
        k_rows = H + q_rows
        v_rows = 2 * H + q_rows
        qk_rows = np.r_[q_rows, k_rows]
        w1t_qk = _tile_rows(w_qkv[qk_rows, :].T)
        w1t_v = _tile_rows(w_qkv[v_rows, :].T)
        b_qk = np.ascontiguousarray(
            b_qkv[qk_rows].reshape(O_QK // P, P).T, dtype=np.float32
        )
        b_v = np.ascontiguousarray(
            np.broadcast_to(b_qkv[v_rows], (P, O_V)), dtype=np.float32
        )
        o_lo = c * O_OUT
        wout_t = _tile_rows(w_out[o_lo : o_lo + O_OUT, :].T)
        b_o = np.ascontiguousarray(
            b_out[o_lo : o_lo + O_OUT].reshape(O_OUT // P, P).T,
            dtype=np.float32,
        )
        in_maps.append(
            {
                "xt": x,
                "w1t_qk": w1t_qk,
                "w1t_v": w1t_v,
                "b_qk": b_qk,
                "b_v": b_v,
                "wout_t": wout_t,
                "b_out": b_o,
                "ones_c": np.ones((P, 1), dtype=ml_dtypes.bfloat16),
                "ones_r": np.ones((1, P), dtype=ml_dtypes.bfloat16),
            }
        )
    return in_maps


_program_cache = {}


def _get_program(seq=S, mm_dt=MM_DT):
    key = (seq, mm_dt)
    if key not in _program_cache:
        _program_cache[key] = _build_program(seq, mm_dt)
    return _program_cache[key]


def run(hidden_states, w_qkv, b_qkv, w_out, b_out, trace=False, mm_dt=MM_DT):
    """Run the sharded kernel; returns (output, BassKernelResults)."""
    b, s, _ = hidden_states.shape
    nc = _get_program(s, mm_dt)
    in_maps = _make_in_maps(hidden_states, w_qkv, b_qkv, w_out, b_out)
    res = run_bass_kernel_spmd(
        nc, in_maps, list(range(N_CORES)), trace=trace
    )
    # per-core output is out^T [O_OUT, T]; stack to [H, T] then transpose
    cols = np.concatenate([res.results[c]["out"] for c in range(N_CORES)], axis=0)
    return (
        np.ascontiguousarray(cols.T).reshape(b, s, H).astype(np.float32),
        res,
    )


def kernel(hidden_states, w_qkv, b_qkv, w_out, b_out):
    out, _ = run(
        np.asarray(hidden_states),
        np.asarray(w_qkv),
        np.asarray(b_qkv),
        np.asarray(w_out),
        np.asarray(b_out),
    )
    return out
